# revision 31
# baseline (speedup 1.0000x reference)
"""Multi-head attention layer (QKV proj + RoPE + SDPA + o_proj) on 8 TRN2 cores.

Sharding: DP2 x TP4. Core c handles batch c//4 and heads 4*(c%4)..4*(c%4)+4.
Each core computes its 4 heads' attention and a partial o_proj output
[L, D]; the host sums the 4 partials per batch (row-parallel o_proj).

All matmul operands are bf16 (same 1 cycle/row PE rate as fp32r on TRN2,
half the DMA/SBUF footprint); PSUM accumulation is fp32 throughout.

Structure (single PE stream, minimal gaps). Phase 1 runs TAG-MAJOR (all
q projections, then all k, then all v) so that by the time the v
projections stream, qT/kT are fully roped and the first attention
score+exp supersteps can interleave into the v-pass — the ACT exp
stream (the binding engine of the attention phase, ~1.1us per
[128,1024] exp) gets an 8-step head start on otherwise-idle ACT time.
hidden_states is re-streamed from DRAM per pass (3x8MB, far under DMA
capacity); wq/wk/wv share two rotating SBUF slots so the v weights
land during the k-pass without reserving a third 16KB/partition slot.

  q/k pass: for each 512-token block n, stream htk and run 16 matmuls
           per kg into 4 PSUM accumulation chains (m-major within kg so
           single-buffered banks flush in time). RoPE is PE-free: the
           rotate-half permutation is two ACT partition-offset copies
           (sign folded into sin host-side), cos/sin multiplies on DVE
           in bf16 (2x mode), final add on GpSimd. Rope units are
           deferred one kg boundary so no engine queue piles up.
  v pass:  one 16-matmul chain per 128-token column, ping-ponging 2
           PSUM banks; after every other chain one attention front
           (2 score matmuls + 1 exp) is interleaved.
  attention: 2-kv-tile supersteps; av trails scores by 8 steps (the
           v-pass head start), probs buffered 10 deep. Scores land in a
           [128,1024] fp32 PSUM tile (two single-shot 512-wide matmuls
           -> one 1024-wide exp; wq carries the 1/sqrt(Hd) scale).
           Softmax skips max-subtraction (scores ~N(0,1)). Denominator:
           probs accumulate into 2 bf16 partials on DVE (combined on
           DVE at unit end — never a GpSimd hop, whose multi-us op
           latency would gate the in-order PE queue); the PE only does
           2 single-shot ones-matmuls per (head, half), written into
           ps_out[0:1] after its copy-out, emitted 3 supersteps late so
           the in-order PE queue never waits on them. ps_out is
           double-buffered so the next unit accumulates while the tail
           (DVE reciprocal -> gpsimd.partition_broadcast -> DVE
           normalize) drains.
  o_proj:  each (token-tile, d-block-pair) quantum runs its two
           4-matmul accumulation chains in a PSUM slot borrowed
           round-robin from the attention pools (8 banks in rotation),
           starting while the last attention tails drain. bf16 output
           DMA (summed to fp32 on the host); the last token tile DMAs
           per-d-block to shorten the drain.

Accumulation-chain rule learned the hard way: `start=True` clears the
has_written bits for the WHOLE PSUM bank, so two interleaved multi-step
accumulation chains must never share a bank (single-shot matmuls may).
"""

import numpy as np

import sys
import types

# Defensive: concourse.bass_utils imports antenv.axon_hooks when tracing is
# requested; provide a null shim if the module is absent in this image so a
# stray BASS_TRACE env var cannot crash the kernel.
try:
    import antenv.axon_hooks  # noqa: F401
except ImportError:
    _m = types.ModuleType("antenv.axon_hooks")
    _m.set_axon_ntff_profile_hook = lambda h: None
    _m.get_axon_ntff_profile_hook = lambda: None
    sys.modules["antenv.axon_hooks"] = _m

import ml_dtypes

import concourse.bass as bass
import concourse.mybir as mybir
import concourse.tile as tile
from concourse import bacc
from concourse.bass_utils import run_bass_kernel_spmd

# problem constants (hardcoded per spec)
B, L, D = 2, 2048, 2048
H, Hd = 16, 128
NC = 8
TPH = 4            # heads per core
QKV = TPH * Hd     # 512 per-core projection width
KT = D // 128      # 16 contraction tiles
NT = L // 512      # 4 token groups of 512
MT = L // 128      # 16 token chunks of 128

f32 = mybir.dt.float32
bf16 = mybir.dt.bfloat16

AF = mybir.ActivationFunctionType
SCALE = 1.0 / float(np.sqrt(Hd))

_CACHE: dict = {}


def _build():
    nc = bacc.Bacc("TRN2", target_bir_lowering=False, debug=False)

    # inputs are pre-tiled on the host so every DMA line is contiguous per
    # partition (4-16KB instead of 1KB)
    hTt = nc.dram_tensor("hTt", [NT, 128, KT, 512], bf16, kind="ExternalInput").ap()
    wqT = nc.dram_tensor("wqT", [128, KT, QKV], bf16, kind="ExternalInput").ap()
    wkT = nc.dram_tensor("wkT", [128, KT, QKV], bf16, kind="ExternalInput").ap()
    wvT = nc.dram_tensor("wvT", [128, KT, QKV], bf16, kind="ExternalInput").ap()
    woT = nc.dram_tensor("woT", [128, TPH, D], bf16, kind="ExternalInput").ap()
    cosT = nc.dram_tensor("cosT", [Hd, L], bf16, kind="ExternalInput").ap()
    sinTs = nc.dram_tensor("sinTs", [Hd, L], bf16, kind="ExternalInput").ap()
    out = nc.dram_tensor("out", [L, D], bf16, kind="ExternalOutput").ap()

    out_re = out.rearrange("(mm p) (nb d) -> p mm nb d", p=128, d=512)

    HW = 1024   # tq half-width
    F = 8       # attention fronts interleaved into the v-pass (= av lag)

    with tile.TileContext(nc) as tc:
        with tc.tile_pool(name="persist", bufs=1) as persist:
            # ---- persistent tensors -----------------------------------
            ones_b = persist.tile([128, 1], bf16, name="ones_b")
            nc.vector.memset(ones_b, 1.0)
            warm = persist.tile([128, 512], bf16, name="warm")
            nc.vector.memset(warm, 0.0)
            qT = [persist.tile([Hd, L], bf16, name=f"qT{h}") for h in range(TPH)]
            kT = [persist.tile([Hd, L], bf16, name=f"kT{h}") for h in range(TPH)]
            v_big = persist.tile([128, MT, QKV], bf16, name="v_big")
            outT = [persist.tile([Hd, L], bf16, name=f"outT{h}") for h in range(TPH)]
            cos_sb = persist.tile([Hd, L], bf16, name="cos_sb")
            sin_sb = persist.tile([Hd, L], bf16, name="sin_sb")

            with (
                tc.tile_pool(name="wsh", bufs=2) as wsh,
                tc.tile_pool(name="stream", bufs=2) as stream,
                tc.tile_pool(name="tmp", bufs=2) as tmp,
                tc.tile_pool(name="wo", bufs=1) as wop,
                tc.tile_pool(name="att", bufs=2) as att,
            ):
                # ---- prefetch wave: first htk block + q/k weights ------
                htk0 = stream.tile([128, KT, 512], bf16, name="htk")
                for kg in range(4):
                    nc.sync.dma_start(
                        out=htk0[:, kg * 4 : (kg + 1) * 4, :],
                        in_=hTt[0, :, kg * 4 : (kg + 1) * 4, :],
                    )
                # w slots rotate: q -> slot A, k -> slot B, v -> slot A
                # again (the v DMA naturally waits for the q-pass readers)
                w_q = wsh.tile([128, KT, QKV], bf16, name="w")
                for a, b in (
                    (0, 1), (1, 2), (2, 4), (4, 6), (6, 8),
                    (8, 10), (10, 12), (12, 16),
                ):
                    nc.gpsimd.dma_start(out=w_q[:, a:b, :], in_=wqT[:, a:b, :])
                # w_k is allocated now but its DMA is deferred into the
                # q-pass (n=1/n=2): the first ~13us are DMA-bandwidth-bound
                # on htk0+wq+cos/sin, and wk isn't consumed until the
                # k-pass (~68us in)
                w_k = wsh.tile([128, KT, QKV], bf16, name="w")
                # rope constants: first half needed at the first flush
                nc.sync.dma_start(out=cos_sb[:, 0:HW], in_=cosT[:, 0:HW])
                nc.sync.dma_start(out=sin_sb[:, 0:HW], in_=sinTs[:, 0:HW])

                # deferred rope work: list of closures, emitted one per kg
                # boundary of the *following* projection stream so the ACT
                # queue (which also drains the PSUM flush copies) never
                # backs up.
                pending_rope = []

                def emit_one_rope():
                    if pending_rope:
                        pending_rope.pop(0)()

                def make_rope(dst, m, n, raw):
                    csl = slice(n * 512, (n + 1) * 512)

                    def do():
                        # rotate-half permutation: two partition-offset
                        # copies on ACT (sign already folded into sinTs)
                        rr = tmp.tile([128, 512], bf16, name="rr")
                        nc.scalar.copy(rr[0:64, :], raw[64:128, :])
                        nc.scalar.copy(rr[64:128, :], raw[0:64, :])
                        t1 = tmp.tile([128, 512], bf16, name="t1")
                        nc.vector.tensor_mul(t1, raw, cos_sb[:, csl])
                        t2 = tmp.tile([128, 512], bf16, name="t2")
                        nc.vector.tensor_mul(t2, rr, sin_sb[:, csl])
                        nc.gpsimd.tensor_add(dst[m][:, csl], t1, t2)

                    return do

                # ==== q / k passes ======================================
                with tc.tile_pool(name="pp", bufs=1, space="PSUM") as pp:
                    # PE warm-up: junk matmuls during the initial DMA wait
                    # so the clock ramp completes before real data lands
                    ps_warm = pp.tile([128, 512], f32, name="pp0", bufs=2)
                    for _ in range(7):
                        nc.tensor.matmul(
                            ps_warm, warm[:, 0:128], warm, start=True, stop=True
                        )

                    def qk_pass(w_sb, dst, first):
                        for n in range(NT):
                            if first and n == 0:
                                htk = htk0
                            else:
                                htk = stream.tile([128, KT, 512], bf16, name="htk")
                                for kg in range(2):
                                    nc.sync.dma_start(
                                        out=htk[:, kg * 8 : (kg + 1) * 8, :],
                                        in_=hTt[n, :, kg * 8 : (kg + 1) * 8, :],
                                    )
                            if first and n == 1:
                                # second half of the rope constants
                                nc.sync.dma_start(
                                    out=cos_sb[:, HW:L], in_=cosT[:, HW:L]
                                )
                                nc.sync.dma_start(
                                    out=sin_sb[:, HW:L], in_=sinTs[:, HW:L]
                                )
                            if first and n in (1, 2):
                                # deferred wk prefetch, clear of the start
                                # window
                                a = (n - 1) * 8
                                nc.gpsimd.dma_start(
                                    out=w_k[:, a : a + 8, :],
                                    in_=wkT[:, a : a + 8, :],
                                )
                            ps_x = [
                                pp.tile(
                                    [128, 512], f32, name=f"pp{m}",
                                    bufs=2 if m < 2 else 1,
                                )
                                for m in range(TPH)
                            ]
                            # stationary = weight m-tile, moving = htk.
                            # m-major within each kg so the first matmul of
                            # m2/m3 (single-buffered banks) comes late
                            # enough for the previous flush to have freed
                            # them.
                            for kg in range(4):
                                for m in range(TPH):
                                    for i in range(4):
                                        kk = kg * 4 + i
                                        nc.tensor.matmul(
                                            ps_x[m],
                                            w_sb[:, kk, m * 128 : (m + 1) * 128],
                                            htk[:, kk, :],
                                            start=(kk == 0),
                                            stop=(kk == KT - 1),
                                        )
                                emit_one_rope()
                            # flush: raw copies split ACT/DVE, rope deferred
                            raws = []
                            for m in range(TPH):
                                raw = tmp.tile([128, 512], bf16, name="raw", bufs=4)
                                if m < 2:
                                    nc.scalar.copy(raw, ps_x[m])
                                else:
                                    nc.vector.tensor_copy(raw, ps_x[m])
                                raws.append(raw)
                            for m in range(TPH):
                                pending_rope.append(make_rope(dst, m, n, raws[m]))

                    qk_pass(w_q, qT, True)
                    # v weights into slot A (waits q-pass readers via the
                    # tile framework; by the time the gpsimd queue reaches
                    # these DGE ops the q-pass is done, so nothing blocks),
                    # plus o_proj weights on the sync queue
                    w_v = wsh.tile([128, KT, QKV], bf16, name="w")
                    for a, b in ((0, 4), (4, 8), (8, 12), (12, 16)):
                        nc.gpsimd.dma_start(out=w_v[:, a:b, :], in_=wvT[:, a:b, :])
                    qk_pass(w_k, kT, False)

                # ==== attention machinery ===============================
                wo_sb = wop.tile([128, TPH, D], bf16, name="wo_sb")

                seq = [
                    (half, h, tk)
                    for half in range(2)
                    for h in range(TPH)
                    for tk in range(MT)
                ]
                n_seq = len(seq)
                state = {}  # (half,h) -> ps_out
                probs_by_idx = {}
                partials = {}
                # partial index by tk: p0 = tk0-7 and tk15, p1 = tk8-14
                # (all on DVE). tk15 routes to p0 and the p0+=p1 combine
                # runs at tk14, so the den matmul waits on one DVE op
                # fewer after the last probs tile.
                P_OF_TK = [0] * 8 + [1] * 7 + [0]

                with tc.tile_pool(name="pss", bufs=2, space="PSUM") as pss:
                    def front(t):
                        half, h, tk = seq[t]
                        # full-width scores tile (2 banks); each 512-wide
                        # matmul is single-shot (start&stop) into its own
                        # bank. One 1024-wide exp serves both. wq is
                        # pre-scaled by 1/sqrt(Hd) on the host.
                        sc_ps = pss.tile([128, HW], f32, name="sc")
                        for j in range(2):
                            tq0 = half * HW + j * 512
                            nc.tensor.matmul(
                                sc_ps[:, j * 512 : (j + 1) * 512],
                                kT[h][:, tk * 128 : (tk + 1) * 128],
                                qT[h][:, tq0 : tq0 + 512],
                                start=True,
                                stop=True,
                            )
                        probs = att.tile([128, HW], bf16, name="probs", bufs=F + 2)
                        probs_by_idx[t] = probs
                        nc.scalar.activation(probs, sc_ps, AF.Exp)

                    # ==== v pass, with the first F fronts interleaved ====
                    with tc.tile_pool(name="pv", bufs=2, space="PSUM") as pv:
                        next_front = [0]
                        for n in range(NT):
                            htk = stream.tile([128, KT, 512], bf16, name="htk")
                            for kg in range(2):
                                nc.sync.dma_start(
                                    out=htk[:, kg * 8 : (kg + 1) * 8, :],
                                    in_=hTt[n, :, kg * 8 : (kg + 1) * 8, :],
                                )
                            if n == 0:
                                for hh in range(TPH):
                                    nc.sync.dma_start(
                                        out=wo_sb[:, hh, :], in_=woT[:, hh, :]
                                    )
                            for mc in range(4):
                                ps_v = pv.tile([128, 512], f32, name="pv")
                                for kk in range(KT):
                                    nc.tensor.matmul(
                                        ps_v,
                                        htk[:, kk, mc * 128 : (mc + 1) * 128],
                                        w_v[:, kk, :],
                                        start=(kk == 0),
                                        stop=(kk == KT - 1),
                                    )
                                if (n * 4 + mc) % 2 == 0:
                                    nc.scalar.copy(v_big[:, n * 4 + mc, :], ps_v)
                                else:
                                    nc.vector.tensor_copy(
                                        v_big[:, n * 4 + mc, :], ps_v
                                    )
                                emit_one_rope()
                                if (n * 4 + mc) % 4 != 0 and next_front[0] < F:
                                    front(next_front[0])
                                    next_front[0] += 1
                        while pending_rope:
                            emit_one_rope()

                    # ==== main attention loop + o_proj ===================
                    with tc.tile_pool(name="pso", bufs=2, space="PSUM") as pso:
                        def back_av(t):
                            half, h, tk = seq[t]
                            if (half, h) not in state:
                                state[(half, h)] = pso.tile(
                                    [Hd, HW], f32, name="ps_out"
                                )
                            ps_out = state[(half, h)]
                            probs = probs_by_idx.pop(t)
                            st = dict(start=(tk == 0), stop=(tk == MT - 1))
                            for j in range(2):
                                nc.tensor.matmul(
                                    ps_out[:, j * 512 : (j + 1) * 512],
                                    v_big[:, tk, h * 128 : (h + 1) * 128],
                                    probs[:, j * 512 : (j + 1) * 512],
                                    **st,
                                )
                            p_idx = P_OF_TK[tk]
                            parts = partials.setdefault((half, h), [None] * 2)
                            if parts[p_idx] is None:
                                pt = att.tile(
                                    [128, HW], bf16, name=f"part{p_idx}", bufs=2
                                )
                                parts[p_idx] = pt
                                nc.vector.tensor_copy(pt, probs)
                            else:
                                pt = parts[p_idx]
                                nc.vector.tensor_add(pt, pt, probs)
                            if tk == MT - 2:
                                # early combine: p1 is complete at tk14
                                nc.vector.tensor_add(
                                    parts[0], parts[0], parts[1]
                                )

                        def tail_a(t):
                            half, h, tk = seq[t]
                            ps_out = state[(half, h)]
                            # copy out the unnormalized attention output (on
                            # DVE; the den matmuls wait for this WAR plus
                            # tk15's partial add only)
                            sl = slice(half * HW, (half + 1) * HW)
                            nc.vector.tensor_copy(outT[h][:, sl], ps_out)

                        def tail_b(t):
                            half, h, tk = seq[t]
                            ps_out = state.pop((half, h))
                            parts = partials.pop((half, h))
                            # denominator row: 2 single-shot ones-matmuls
                            # into ps_out[0:1] (bank already copied out by
                            # tail_a, so no dedicated PSUM banks for the
                            # denominator)
                            den_ap = ps_out[0:1, :]
                            for j in range(2):
                                nc.tensor.matmul(
                                    den_ap[:, j * 512 : (j + 1) * 512],
                                    ones_b,
                                    parts[0][:, j * 512 : (j + 1) * 512],
                                    start=True,
                                    stop=True,
                                )
                            # fast reciprocal on DVE, partition broadcast on
                            # GpSimd, in-place DVE normalize
                            sl = slice(half * HW, (half + 1) * HW)
                            rec = att.tile([1, HW], f32, name="rec", bufs=1)
                            nc.vector.reciprocal_approx_fast(rec, den_ap)
                            rec_bc = att.tile([128, HW], f32, name="rec_bc", bufs=1)
                            nc.gpsimd.partition_broadcast(rec_bc, rec, channels=128)
                            nc.vector.tensor_mul(
                                outT[h][:, sl], outT[h][:, sl], rec_bc
                            )

                        # ---- o_proj work quanta ---------------------------
                        ot4_by_m = {}
                        oproj_copy_n = [0]

                        def oproj_quantum(m, pair, engines, pool, tname):
                            of = pool.tile([128, HW], f32, name=tname)
                            if pair == 0:
                                ot4_by_m[m] = att.tile(
                                    [128, 4, 512], bf16, name="ot4", bufs=2
                                )
                            ot4 = ot4_by_m[m]
                            for nblk in (2 * pair, 2 * pair + 1):
                                bsl = slice(
                                    (nblk % 2) * 512, (nblk % 2) * 512 + 512
                                )
                                for hh in range(TPH):
                                    nc.tensor.matmul(
                                        of[:, bsl],
                                        outT[hh][:, m * 128 : (m + 1) * 128],
                                        wo_sb[:, hh, nblk * 512 : (nblk + 1) * 512],
                                        start=(hh == 0),
                                        stop=(hh == TPH - 1),
                                    )
                            for nblk in (2 * pair, 2 * pair + 1):
                                bsl = slice(
                                    (nblk % 2) * 512, (nblk % 2) * 512 + 512
                                )
                                eng = engines[oproj_copy_n[0] % len(engines)]
                                oproj_copy_n[0] += 1
                                if eng == "v":
                                    nc.vector.tensor_copy(
                                        ot4[:, nblk, :], of[:, bsl]
                                    )
                                else:
                                    nc.scalar.copy(ot4[:, nblk, :], of[:, bsl])
                            if pair == 1:
                                ot4 = ot4_by_m.pop(m)
                                if m == MT - 1:
                                    # last tile: per-d-block DMAs to shorten
                                    # the final drain
                                    for nblk in range(4):
                                        nc.sync.dma_start(
                                            out=out_re[:, m, nblk, :],
                                            in_=ot4[:, nblk, :],
                                        )
                                else:
                                    nc.sync.dma_start(
                                        out=out_re[:, m, :, :], in_=ot4
                                    )

                        # supersteps: sc pairs for (s, s+1), av pairs
                        # trailing by F. A unit's tail is split: tail_a
                        # (DVE copy-out + partial combine) runs with its
                        # superstep; tail_b (den matmuls + recip +
                        # broadcast + normalize) is delayed 3 supersteps so
                        # the den matmul reaches the front of the in-order
                        # PE queue only after its DVE dependencies have
                        # resolved.
                        pending_tail_b = []  # (due_superstep, t)
                        for s in range(F, n_seq + F + 8, 2):
                            for t in (s, s + 1):
                                if t < n_seq:
                                    front(t)
                            for t in (s - F, s - F + 1):
                                if 0 <= t < n_seq:
                                    back_av(t)
                            while pending_tail_b and pending_tail_b[0][0] <= s:
                                tail_b(pending_tail_b.pop(0)[1])
                            for t in (s - F, s - F + 1):
                                if 0 <= t < n_seq and seq[t][2] == MT - 1:
                                    tail_a(t)
                                    pending_tail_b.append((s + 6, t))
                        # o_proj: rotate quanta over the four freed PSUM
                        # slot groups; copies alternate ACT/DVE now that
                        # the exp stream is done
                        slot_cycle = [
                            (pss, "sc"), (pss, "sc"),
                            (pso, "ps_out"), (pso, "ps_out"),
                        ]
                        qn = 0
                        for m in range(MT):
                            for pair in range(2):
                                pool, tname = slot_cycle[qn % 4]
                                qn += 1
                                oproj_quantum(m, pair, ("s", "v"), pool, tname)

    nc.compile()
    return nc


def _bf(x: np.ndarray) -> np.ndarray:
    return np.ascontiguousarray(x, dtype=np.float32).astype(ml_dtypes.bfloat16)


def kernel(hidden_states, cos, sin, wq, wk, wv, wo):
    if "nc" not in _CACHE:
        _CACHE["nc"] = _build()
    nc = _CACHE["nc"]

    hidden_states = np.asarray(hidden_states, dtype=np.float32)
    cos = np.asarray(cos, dtype=np.float32)
    sin = np.asarray(sin, dtype=np.float32)
    wq = np.asarray(wq, dtype=np.float32)
    wk = np.asarray(wk, dtype=np.float32)
    wv = np.asarray(wv, dtype=np.float32)
    wo = np.asarray(wo, dtype=np.float32)

    # host-side layout prep
    cosT = _bf(cos[0, 0].T)                             # [Hd, L]
    sinT = np.ascontiguousarray(sin[0, 0].T)            # [Hd, L]
    sinTs = sinT.copy()
    sinTs[: Hd // 2] *= -1.0                            # fold rotate_half signs
    sinTs = _bf(sinTs)

    # pre-tile for contiguous per-partition DMA lines:
    #   hTt[n, p, kk, t] = h.T[kk*128+p, n*512+t]
    #   w*T[p, kk, r]    = w[r, kk*128+p]   (w.T row d = kk*128+p)
    #   woT[p, hh, d]    = wo[d, r0+hh*128+p]
    def _tile_h(hb):
        return _bf(
            hb.T.reshape(KT, 128, NT, 512).transpose(2, 1, 0, 3)
        )

    def _tile_w(wrows):
        return _bf(wrows.T.reshape(KT, 128, QKV).transpose(1, 0, 2))

    def _tile_wo(wcols):
        return _bf(wcols.T.reshape(TPH, 128, D).transpose(1, 0, 2))

    hTt = [_tile_h(hidden_states[b]) for b in range(B)]
    # fold the attention scale into wq so raw scores are ~N(0,1) (keeps the
    # bf16 PSUM score rounding relative) and exp() needs no scale
    wq = wq * SCALE

    in_maps = []
    for c in range(NC):
        b = c // 4
        hb = c % 4
        r0 = hb * QKV
        in_maps.append(
            {
                "hTt": hTt[b],
                "wqT": _tile_w(wq[r0 : r0 + QKV]),
                "wkT": _tile_w(wk[r0 : r0 + QKV]),
                "wvT": _tile_w(wv[r0 : r0 + QKV]),
                "woT": _tile_wo(wo[:, r0 : r0 + QKV]),
                "cosT": cosT,
                "sinTs": sinTs,
            }
        )

    res = run_bass_kernel_spmd(nc, in_maps, core_ids=list(range(NC)))
    _CACHE["last_results"] = res

    out = np.zeros((B, L, D), dtype=np.float32)
    for c in range(NC):
        out[c // 4] += np.asarray(res.results[c]["out"], dtype=np.float32)
    return out
